# revision 1
# baseline (speedup 1.0000x reference)
"""GCN message-passing kernel for Trainium2 (8 NeuronCores, Bass/Tile).

Math (reference): h = x @ W; msg = h[src] * w_e; agg = segment_sum(msg, dst);
y = BETA*z + (C-BETA)*relu(z) with z = agg + b.

We use linearity to aggregate x first: agg_x = segment_sum(x[src] * w_e, dst),
then y = f(agg_x @ W + b). Per-core dst-sharding (12500 nodes each); edges are
routed to the dst-owner core on the host.

Device algorithm per core:
  - Edges sorted by (dst_block, src_quartile), padded per (block, quartile)
    to multiples of 128 (dummy edges have w=0), with a SHARED (across cores)
    static chunk schedule K[block][quartile] = max over cores.
  - dma_gather pulls x[src] rows (512B) into SBUF tiles M [128 edge, 128 feat]
    (gpsimd custom op; int16 indices limited to 32767 -> 4 src-range tables).
  - One-hot matmul per 128-edge chunk: ST[e, d] = (iota_d == dst_e) * w_e
    built in ONE vector op (tensor_scalar is_equal+mult with per-partition
    operands); PSUM accumulates aggT[feat, dst_block] += M.T @ ST.
  - Per block: aggT -> SBUF, z.T = W.T @ aggT (+bias), y = BETA*z +
    (C-BETA)*relu(z) via one ACT + two DVE ops, DMA to the transposed
    output yT [128, 12544]. Host transposes back and trims.
"""
import numpy as np
from contextlib import ExitStack

N_NODES = 100000
N_EDGES = 1600000
D = 128
P = 8
NBLK = 100                     # blocks per core (128 dst slots each)
NDST = NBLK * 128              # 12800 dst slots per core
NCLS = 4
CLSZ = N_NODES // NCLS         # 25000
SEGBLK = 5
BETA = 0.5
C_CONST = 1.0
NBINS = P * NBLK               # 800
NTOT = NBINS * 128             # 102400 slots (2400 dummies)


def _balance_dst(deg):
    """Assign dst nodes (+dummies) to 800 (core, block) bins of 128 slots,
    balancing per-class loads toward <=512 so the shared chunk schedule is
    uniform (Kbq=4). Returns binof[NTOT], slotof[NTOT]."""
    LIMIT = 512
    tot = deg.sum(1)
    order = np.argsort(-tot, kind="stable")
    assign = np.empty(NTOT, np.int32)
    fwd = np.arange(NBINS, dtype=np.int32)
    for r in range(128):
        idx = order[r * NBINS : (r + 1) * NBINS]
        assign[idx] = fwd if r % 2 == 0 else fwd[::-1]

    members = np.argsort(assign, kind="stable").reshape(NBINS, 128)
    L = np.zeros((NBINS, NCLS), np.int64)
    np.add.at(L, assign, deg)

    for _ in range(200):
        worst = int((L - LIMIT).max())
        if worst <= 0:
            break
        for q in range(NCLS):
            ob = np.where(L[:, q] > LIMIT)[0]
            if len(ob) == 0:
                continue
            ob = ob[np.argsort(-L[ob, q])]
            ub = np.argsort(L[:, q], kind="stable")
            ub = ub[L[ub, q] < LIMIT - 8][: len(ob)]
            for bo, bu in zip(ob, ub):
                mo, mu = members[bo], members[bu]
                j1 = int(np.argmax(deg[mo, q]))
                j2 = int(np.argmin(deg[mu, q]))
                n1, n2 = mo[j1], mu[j2]
                if deg[n1, q] <= deg[n2, q]:
                    continue
                members[bo, j1], members[bu, j2] = n2, n1
                ddelta = deg[n1] - deg[n2]
                L[bo] -= ddelta
                L[bu] += ddelta

    # Per-core block-label alignment: sort each core's bins by chunk-need
    # profile so residual >512 groups coincide across cores (shared max).
    need = -((-L) // 128)  # ceil(L/128)
    binof = np.empty(NTOT, np.int64)
    slotof = np.empty(NTOT, np.int64)
    neworder = np.empty(NBINS, np.int64)
    for c in range(P):
        rows = np.arange(c * NBLK, (c + 1) * NBLK)
        key = np.lexsort(
            (L[rows, 3], L[rows, 2], L[rows, 1], L[rows, 0],
             need[rows, 3], need[rows, 2], need[rows, 1], need[rows, 0])
        )
        neworder[rows] = rows[key]
    members = members[neworder]
    for b in range(NBINS):
        binof[members[b]] = b
        slotof[members[b]] = np.arange(128)
    return binof, slotof


def _plan_and_pack(src, dst, w):
    """Host-side routing. Returns the static schedule and per-core arrays."""
    src = np.ascontiguousarray(src.astype(np.int64))
    dst = np.ascontiguousarray(dst.astype(np.int64))
    w = np.ascontiguousarray(w.astype(np.float32))

    cls = src // CLSZ
    deg = np.zeros((NTOT, NCLS), np.int64)
    np.add.at(deg, (dst, cls), 1)
    binof, slotof = _balance_dst(deg)

    dbin = binof[dst]
    core = dbin // NBLK
    blk = dbin % NBLK
    d128 = slotof[dst].astype(np.float32)
    idxl = (src - cls * CLSZ).astype(np.int16)

    key = ((core * NBLK + blk) * NCLS + cls).astype(np.int64)
    order = np.argsort(key, kind="stable")
    key_s = key[order]

    cnt = np.bincount(key, minlength=P * NBLK * NCLS).reshape(P, NBLK, NCLS)
    Kbq = (cnt + 127) // 128
    Kbq = Kbq.max(axis=0)                      # [NBLK, NCLS] shared schedule
    Kbq[:, 0] = np.maximum(Kbq[:, 0], 1)       # ensure start=True exists
    Kb = Kbq.sum(axis=1)                       # chunks per block
    C = int(Kb.sum())                          # total chunks per core

    # segments of SEGBLK blocks; within a segment, chunk columns are laid out
    # class-major: [q=0: all blocks' chunks][q=1: ...]... (gather-call order)
    seg_of_blk = np.arange(NBLK) // SEGBLK
    nseg = int(seg_of_blk.max()) + 1
    col_of = np.zeros((NBLK, NCLS), np.int64)  # first chunk col of (b, q)
    seg_base = np.zeros(nseg + 1, np.int64)    # first col of each segment
    segs = []                                  # per segment: (b0, b1, [(q, colbase, nchunks)])
    colp = 0
    for s in range(nseg):
        b0, b1 = s * SEGBLK, min((s + 1) * SEGBLK, NBLK)
        seg_base[s] = colp
        calls = []
        for q in range(NCLS):
            callbase = colp
            for b in range(b0, b1):
                col_of[b, q] = colp
                colp += int(Kbq[b, q])
            if colp > callbase:
                calls.append((q, callbase, colp - callbase))
        segs.append((b0, b1, calls))
    seg_base[nseg] = colp
    assert colp == C

    # group start of each (core, blk, cls) in the sorted edge list
    cntf = cnt.reshape(-1)
    gstart = np.zeros(P * NBLK * NCLS, np.int64)
    np.cumsum(cntf[:-1], out=gstart[1:])
    grank = np.arange(N_EDGES, dtype=np.int64) - gstart[key_s]

    co = core[order]
    bo = blk[order]
    qo = cls[order]
    chunkpos = col_of[bo, qo] + (grank >> 7)   # global chunk column
    lane = (grank & 127).astype(np.int64)

    dsel = np.zeros((P, 128, C), np.float32)
    wts = np.zeros((P, 128, C), np.float32)
    idxw16 = np.zeros((P, 16, C * 8), np.int16)

    dsel[co, lane, chunkpos] = d128[order]
    wts[co, lane, chunkpos] = w[order]
    idxw16[co, lane & 15, chunkpos * 8 + (lane >> 4)] = idxl[order]

    idxw = np.tile(idxw16, (1, 8, 1))          # replicate to 128 partitions

    # block-major column order for dsel/wts (ST batch build): all chunks of a
    # block contiguous. idxw keeps the (segment, class)-major gather order.
    perm = np.empty(C, np.int64)
    blk_base = np.zeros(NBLK + 1, np.int64)
    p = 0
    for b in range(NBLK):
        blk_base[b] = p
        for q in range(NCLS):
            for i in range(int(Kbq[b, q])):
                perm[p] = col_of[b, q] + i
                p += 1
    blk_base[NBLK] = p
    dsel = dsel[:, :, perm]
    wts = wts[:, :, perm]

    meta = {
        "C": C,
        "Kbq": Kbq,
        "blk_base": blk_base,
        "col_of": col_of,
        "seg_base": seg_base,
        "segs": segs,
        "kseg_max": int((seg_base[1:] - seg_base[:-1]).max()),
        "binof": binof,
        "slotof": slotof,
    }
    return meta, dsel, wts, idxw


def unshard_output(yts, meta):
    """yts: list of per-core yt arrays [128, NBLK*128] -> full y [N_NODES, D]."""
    binof = meta["binof"][:N_NODES]
    slotof = meta["slotof"][:N_NODES]
    allyt = np.stack(yts)                      # [P, 128, NBLK*128]
    core = binof // NBLK
    col = (binof % NBLK) * 128 + slotof
    return np.ascontiguousarray(allyt[core, :, col])


def _build_nc(meta, reps=1, skip_gather=False, skip_st=False, skip_mm=False,
              skip_final=False, st_bf16=True, st_any=True, nq=4, sp=False,
              st_bufs=6, g_bufs=5, meta_bf16=False, fuse_final=True, plain_dma=False,
              st_batch=True, gather_prep=False):
    import concourse.bacc as bacc
    import concourse.tile as tile
    import concourse.mybir as mybir
    from drainfix_embedded import fix_excess_waits

    C = meta["C"]
    Kbq = meta["Kbq"]
    blk_base = meta["blk_base"]
    KBmax = int(Kbq.sum(axis=1).max())
    col_of = meta["col_of"]
    seg_base = meta["seg_base"]
    segs = meta["segs"]
    kseg_max = meta["kseg_max"]
    f32 = mybir.dt.float32

    nc = bacc.Bacc("TRN2", target_bir_lowering=False, debug=False, num_devices=P,
                   num_swdge_queues=nq)
    x_dt = mybir.dt.bfloat16 if st_bf16 else f32
    xtab = nc.declare_dram_parameter("xtab", [N_NODES, D], x_dt, isOutput=False)
    meta_dt = mybir.dt.bfloat16 if meta_bf16 else f32
    dsel_d = nc.declare_dram_parameter("dsel", [128, C], meta_dt, isOutput=False)
    wts_d = nc.declare_dram_parameter("wts", [128, C], meta_dt, isOutput=False)
    idxw_d = nc.declare_dram_parameter("idxw", [128, C * 8], mybir.dt.int16, isOutput=False)
    wmat_d = nc.declare_dram_parameter("wmat", [D, D], f32, isOutput=False)
    bb_d = nc.declare_dram_parameter("bb", [D, 1], f32, isOutput=False)   # BETA*b (plain b when fuse_final)
    if fuse_final:
        alf_d = nc.declare_dram_parameter("alf", [D, 1], f32, isOutput=False)  # leaky slope
    if not fuse_final:
        cb_d = nc.declare_dram_parameter("cb", [D, 1], f32, isOutput=False)   # (C-BETA)*b
    iota_dt = mybir.dt.bfloat16 if st_bf16 else f32
    iota_d = nc.declare_dram_parameter("iotaf", [128, 128], iota_dt, isOutput=False)
    yt = nc.declare_dram_parameter("yt", [128, NBLK * 128], f32, isOutput=True)

    with tile.TileContext(nc) as tc:
        with ExitStack() as ctx:
            consts = ctx.enter_context(tc.tile_pool(name="consts", bufs=1))
            metap = ctx.enter_context(tc.tile_pool(name="meta", bufs=1))
            gpool = ctx.enter_context(tc.tile_pool(name="gseg", bufs=g_bufs))
            stp = ctx.enter_context(tc.tile_pool(name="st", bufs=st_bufs))
            tmpp = ctx.enter_context(tc.tile_pool(name="sttmp", bufs=2))
            evp = ctx.enter_context(tc.tile_pool(name="ev", bufs=8))
            yp = ctx.enter_context(tc.tile_pool(name="y", bufs=8))
            pagg = ctx.enter_context(tc.tile_pool(name="pagg", bufs=5, space="PSUM"))
            pz = ctx.enter_context(tc.tile_pool(name="pz", bufs=3, space="PSUM"))

            iota_sb = consts.tile([128, 128], iota_dt)
            nc.sync.dma_start(iota_sb[:], iota_d[:])
            w_sb = consts.tile([128, 128], f32)
            nc.sync.dma_start(w_sb[:], wmat_d[:])
            bb_sb = consts.tile([128, 1], f32)
            nc.sync.dma_start(bb_sb[:], bb_d[:])
            if fuse_final:
                alf_sb = consts.tile([128, 1], f32)
                nc.sync.dma_start(alf_sb[:], alf_d[:])
            if not fuse_final:
                cb_sb = consts.tile([128, 1], f32)
                nc.sync.dma_start(cb_sb[:], cb_d[:])

            gsems = (
                [nc.alloc_semaphore(f"gsem{q}") for q in range(8)]
                if gather_prep else None
            )
            prep_ctr = [0]
            dsel_sb = metap.tile([128, C], meta_dt)
            wts_sb = metap.tile([128, C], meta_dt)
            idxw_sb = metap.tile([128, C * 8], mybir.dt.int16)

            from contextlib import nullcontext
            loop_cm = tc.For_i(0, reps, 1) if reps > 1 else nullcontext()
            with loop_cm:
             for si, (b0, b1, calls) in enumerate(segs):
                c0s = int(seg_base[si])
                c1s = int(seg_base[si + 1])
                nc.sync.dma_start(idxw_sb[:, c0s * 8 : c1s * 8], idxw_d[:, c0s * 8 : c1s * 8])
                nc.sync.dma_start(dsel_sb[:, c0s:c1s], dsel_d[:, c0s:c1s])
                nc.sync.dma_start(wts_sb[:, c0s:c1s], wts_d[:, c0s:c1s])

             for (b0, b1, calls) in segs:
                s0 = int(seg_base[b1 // SEGBLK if b1 % SEGBLK else b1 // SEGBLK - 1])
                segc0 = int(col_of[b0, 0])           # first col of this segment
                kseg = int(sum(n for (_, _, n) in calls))
                g = gpool.tile([128, kseg_max * 128], x_dt, tag="gseg")
                g3 = g[:].rearrange("p (k d) -> p k d", d=D)
                for ci, (q, callbase, nch) in enumerate(calls):
                    if skip_gather:
                        break
                    lc0 = callbase - segc0
                    nidx = nch * 128
                    if plain_dma:
                        nc.sync.dma_start(
                            g3[:, lc0 : lc0 + nch, :],
                            xtab[0 : nch * 128, :].rearrange(
                                "(p a) d -> p a d", a=nch
                            ),
                        )
                        continue
                    if gather_prep:
                        nc.gpsimd.dma_gather(
                            g3[:, lc0 : lc0 + nch, :],
                            xtab[q * CLSZ : (q + 1) * CLSZ, :],
                            idxw_sb[:, callbase * 8 : (callbase + nch) * 8],
                            nidx,
                            nidx,
                            D,
                            single_packet=sp,
                            queue_num=ci % nq,
                            prepare_only=True,
                            sem=gsems[prep_ctr[0] % 8],
                        )
                        prep_ctr[0] += 1
                        nc.gpsimd.trigger_dma(count=None, queue_num=ci % nq)
                    else:
                        nc.gpsimd.dma_gather(
                            g3[:, lc0 : lc0 + nch, :],
                            xtab[q * CLSZ : (q + 1) * CLSZ, :],
                            idxw_sb[:, callbase * 8 : (callbase + nch) * 8],
                            nidx,
                            nidx,
                            D,
                            single_packet=sp,
                            queue_num=ci % nq,
                        )
                for b in range(b0, b1):
                    pa = None if skip_mm else pagg.tile([128, 128], f32, tag="pagg")
                    nchunks_b = int(Kbq[b].sum())
                    st_dt = mybir.dt.bfloat16 if st_bf16 else f32
                    if st_batch and not skip_st:
                        kb0 = int(blk_base[b])
                        kbn = int(blk_base[b + 1]) - kb0
                        stt = stp.tile([128, KBmax * 128], st_dt, tag="st")
                        st3 = stt[:].rearrange("p (k d) -> p k d", d=128)
                        tmp = tmpp.tile([128, KBmax * 128], st_dt, tag="sttmp")
                        tmp3 = tmp[:].rearrange("p (k d) -> p k d", d=128)
                        iota_bc = iota_sb[:].unsqueeze(1).broadcast_to(
                            [128, kbn, 128]
                        )
                        dsel_bc = dsel_sb[:, kb0 : kb0 + kbn].unsqueeze(
                            2
                        ).broadcast_to([128, kbn, 128])
                        wts_bc = wts_sb[:, kb0 : kb0 + kbn].unsqueeze(
                            2
                        ).broadcast_to([128, kbn, 128])
                        nc.vector.tensor_tensor(
                            tmp3[:, :kbn, :], iota_bc, dsel_bc,
                            op=mybir.AluOpType.is_equal,
                        )
                        nc.vector.tensor_tensor(
                            st3[:, :kbn, :], tmp3[:, :kbn, :], wts_bc,
                            op=mybir.AluOpType.mult,
                        )
                    done = 0
                    for q in range(NCLS):
                        for i in range(int(Kbq[b, q])):
                            cg = int(col_of[b, q]) + i   # global col
                            cl = cg - segc0              # col in segment tile
                            if skip_st:
                                st = iota_sb[:]
                            elif st_batch:
                                st = st3[:, done, :]
                            else:
                                sti = stp.tile([128, 128], st_dt, tag="st")
                                eng = nc.any if st_any else nc.vector
                                eng.tensor_scalar(
                                    sti[:],
                                    iota_sb[:],
                                    dsel_sb[:, cg : cg + 1],
                                    wts_sb[:, cg : cg + 1],
                                    op0=mybir.AluOpType.is_equal,
                                    op1=mybir.AluOpType.mult,
                                )
                                st = sti[:]
                            if skip_mm:
                                done += 1
                                continue
                            nc.tensor.matmul(
                                out=pa[:],
                                lhsT=g3[:, cl, :],
                                rhs=st,
                                start=(done == 0),
                                stop=(done == nchunks_b - 1),
                            )
                            done += 1
                    if skip_final or skip_mm:
                        continue
                    aggT = evp.tile([128, 128], f32, tag="ev")
                    nc.scalar.copy(aggT[:], pa[:])
                    z = pz.tile([128, 128], f32, tag="pz")
                    nc.tensor.matmul(
                        out=z[:], lhsT=w_sb[:], rhs=aggT[:], start=True, stop=True
                    )
                    if fuse_final:
                        # y = BETA*z + (C-BETA)*relu(z) == Lrelu_{BETA/C'}(z) for
                        # BETA=0.5, C=1.0, with z = agg + b folded in as bias.
                        yb = yp.tile([128, 128], f32, tag="yb")
                        nc.scalar.activation(
                            yb[:],
                            z[:],
                            mybir.ActivationFunctionType.Prelu,
                            bias=bb_sb[:],
                            scale=1.0,
                            alpha=alf_sb[:],
                        )
                    else:
                        t1 = yp.tile([128, 128], f32, tag="t1")
                        nc.scalar.activation(
                            t1[:],
                            z[:],
                            mybir.ActivationFunctionType.Relu,
                            bias=cb_sb[:],
                            scale=(C_CONST - BETA),
                        )
                        t2 = yp.tile([128, 128], f32, tag="t2")
                        nc.vector.tensor_scalar(
                            t2[:],
                            z[:],
                            BETA,
                            bb_sb[:],
                            op0=mybir.AluOpType.mult,
                            op1=mybir.AluOpType.add,
                        )
                        yb = yp.tile([128, 128], f32, tag="yb")
                        nc.vector.tensor_tensor(
                            yb[:], t1[:], t2[:], op=mybir.AluOpType.add
                        )
                    nc.sync.dma_start(yt[:, b * 128 : (b + 1) * 128], yb[:])

    nc.compile()
    if gather_prep:
        _retarget_dmasw_waits(nc, gsems)
    fix_excess_waits(nc)
    return nc


def _retarget_dmasw_waits(nc, gsems):
    """Consumers of gen_mode==1 gather preps wait on Tile's DMASW lane sems,
    but the transfer completion posts to the prep's sem= semaphore. Lane
    round-robin (i%8) mirrors our gsem assignment, so rewrite every wait on
    DMASW{k}_* to gsem{k} with the same value."""
    import re
    ids = {}
    names = {}
    for k, h in enumerate(gsems):
        ids[k] = h.num
        names[k] = f"gsem{k}"
    pat = re.compile(r"^DMASW(\d)_")
    def walk(block):
        for ins in block.instructions:
            si = getattr(ins, "sync_info", None)
            if si is None:
                continue
            for u in list(si.on_wait) + list(si.on_update):
                m = pat.match(u.ant_name or "")
                if m:
                    k = int(m.group(1))
                    u.id = ids[k]
                    u.ant_name = names[k]
        for sub in getattr(block, "blocks", []) or []:
            walk(sub)
    for fn in nc.m.functions:
        for b in fn.blocks:
            walk(b)


def make_in_maps(dsel, wts, idxw, x, W, b, ncores=P, meta_bf16=False,
                 fuse_final=True, st_bf16=True):
    import ml_dtypes
    x = np.asarray(x, np.float32)
    W = np.asarray(W, np.float32)
    b = np.asarray(b, np.float32)
    x_cast = x.astype(ml_dtypes.bfloat16) if st_bf16 else x
    iota_dt = ml_dtypes.bfloat16 if st_bf16 else np.float32
    iota = np.tile(np.arange(128, dtype=np.float32), (128, 1)).astype(iota_dt)
    meta_dt = ml_dtypes.bfloat16 if meta_bf16 else np.float32
    in_maps = []
    for c in range(ncores):
        m = {
            "xtab": x_cast,
            "dsel": dsel[c].astype(meta_dt),
            "wts": wts[c].astype(meta_dt),
            "idxw": idxw[c],
            "wmat": W,
            "iotaf": iota,
        }
        if fuse_final:
            m["bb"] = b.reshape(D, 1).astype(np.float32)
            m["alf"] = np.full((D, 1), BETA, np.float32)
        else:
            m["bb"] = (BETA * b).reshape(D, 1).astype(np.float32)
            m["cb"] = ((C_CONST - BETA) * b).reshape(D, 1).astype(np.float32)
        in_maps.append(m)
    return in_maps


def kernel(x, edge_index, edge_weight, W, b):
    x = np.asarray(x, np.float32)
    edge_index = np.asarray(edge_index)
    edge_weight = np.asarray(edge_weight, np.float32)
    W = np.asarray(W, np.float32)
    b = np.asarray(b, np.float32)

    meta, dsel, wts, idxw = _plan_and_pack(edge_index[0], edge_index[1], edge_weight)
    nc = _build_nc(meta)

    from concourse.bass_utils import run_bass_kernel_spmd

    in_maps = make_in_maps(dsel, wts, idxw, x, W, b)
    res = run_bass_kernel_spmd(nc, in_maps, list(range(P)))
    return unshard_output([res.results[c]["yt"] for c in range(P)], meta)


# ---------------------------------------------------------------------------
# Embedded walrus workaround (kernel.py must be self-contained): split excess
# sem waits onto preceding NoOps — this walrus build rejects >1 sync wait on
# Drain and on the extended DMA instructions.
import sys as _sys
import types as _types

_dfx_src = '''
import concourse.mybir as mybir

LIMIT_DEFAULT = 1
LIMIT_BY_TYPE = {mybir.InstDrain: 1}


def fix_excess_waits(nc):
    fixed = 0

    def limit_for(ins):
        for t, lim in LIMIT_BY_TYPE.items():
            if isinstance(ins, t):
                return lim
        return LIMIT_DEFAULT

    def walk(block):
        nonlocal fixed
        insts = block.instructions
        i = 0
        while i < len(insts):
            ins = insts[i]
            si = getattr(ins, "sync_info", None)
            lim = limit_for(ins)
            if si is not None and len(si.on_wait) > lim:
                waits = list(si.on_wait)
                excess, keep = waits[:-lim], waits[-lim:]
                pos = i
                for j in range(0, len(excess), LIMIT_DEFAULT):
                    nop = mybir.InstNoOp(name=f"{ins.name}_xw{j}", ins=[], outs=[])
                    nop.engine = ins.engine
                    nop.sync_info = mybir.SyncInfo(
                        on_wait=excess[j : j + LIMIT_DEFAULT], on_update=[]
                    )
                    try:
                        nc.register_instruction(nop)
                    except Exception:
                        pass
                    insts.insert(pos, nop)
                    pos += 1
                    i += 1
                si.on_wait = keep
                fixed += 1
            i += 1
        for sub in getattr(block, "blocks", []) or []:
            walk(sub)

    for fn in nc.m.functions:
        for b in fn.blocks:
            walk(b)
    return fixed
'''

_mod = _types.ModuleType("drainfix_embedded")
exec(_dfx_src, _mod.__dict__)
_sys.modules["drainfix_embedded"] = _mod



# revision 30
# speedup vs baseline: 2.2535x; 2.2535x over previous
"""GCN message-passing kernel for Trainium2 (8 NeuronCores, Bass/Tile).

Math (reference): h = x @ W; msg = h[src] * w_e; agg = segment_sum(msg, dst);
y = BETA*z + (C-BETA)*relu(z) with z = agg + b.

We use linearity to aggregate x first: agg_x = segment_sum(x[src] * w_e, dst),
then y = f(agg_x @ W + b). Per-core dst-sharding (12500 nodes each); edges are
routed to the dst-owner core on the host.

Device algorithm per core:
  - Edges sorted by (dst_block, src_quartile), padded per (block, quartile)
    to multiples of 128 (dummy edges have w=0), with a SHARED (across cores)
    static chunk schedule K[block][quartile] = max over cores.
  - dma_gather pulls x[src] rows (512B) into SBUF tiles M [128 edge, 128 feat]
    (gpsimd custom op; int16 indices limited to 32767 -> 4 src-range tables).
  - One-hot matmul per 128-edge chunk: ST[e, d] = (iota_d == dst_e) * w_e
    built in ONE vector op (tensor_scalar is_equal+mult with per-partition
    operands); PSUM accumulates aggT[feat, dst_block] += M.T @ ST.
  - Per block: aggT -> SBUF, z.T = W.T @ aggT (+bias), y = BETA*z +
    (C-BETA)*relu(z) via one ACT + two DVE ops, DMA to the transposed
    output yT [128, 12544]. Host transposes back and trims.
"""
import numpy as np
from contextlib import ExitStack

N_NODES = 100000
N_EDGES = 1600000
D = 128
P = 8
NBLK = 100                     # blocks per core (128 dst slots each)
NDST = NBLK * 128              # 12800 dst slots per core
NCLS = 4
CLSZ = N_NODES // NCLS         # 25000
SEGBLK = 5
BETA = 0.5
C_CONST = 1.0
NBINS = P * NBLK               # 800
NTOT = NBINS * 128             # 102400 slots (2400 dummies)


def _balance_dst(deg):
    """Assign dst nodes (+dummies) to 800 (core, block) bins of 128 slots,
    balancing per-class loads toward <=512 so the shared chunk schedule is
    uniform (Kbq=4). Returns binof[NTOT], slotof[NTOT]."""
    LIMIT = 512
    tot = deg.sum(1)
    order = np.argsort(-tot, kind="stable")
    assign = np.empty(NTOT, np.int32)
    fwd = np.arange(NBINS, dtype=np.int32)
    for r in range(128):
        idx = order[r * NBINS : (r + 1) * NBINS]
        assign[idx] = fwd if r % 2 == 0 else fwd[::-1]

    members = np.argsort(assign, kind="stable").reshape(NBINS, 128)
    L = np.zeros((NBINS, NCLS), np.int64)
    np.add.at(L, assign, deg)

    for _ in range(200):
        worst = int((L - LIMIT).max())
        if worst <= 0:
            break
        for q in range(NCLS):
            ob = np.where(L[:, q] > LIMIT)[0]
            if len(ob) == 0:
                continue
            ob = ob[np.argsort(-L[ob, q])]
            ub = np.argsort(L[:, q], kind="stable")
            ub = ub[L[ub, q] < LIMIT - 8][: len(ob)]
            for bo, bu in zip(ob, ub):
                mo, mu = members[bo], members[bu]
                j1 = int(np.argmax(deg[mo, q]))
                j2 = int(np.argmin(deg[mu, q]))
                n1, n2 = mo[j1], mu[j2]
                if deg[n1, q] <= deg[n2, q]:
                    continue
                members[bo, j1], members[bu, j2] = n2, n1
                ddelta = deg[n1] - deg[n2]
                L[bo] -= ddelta
                L[bu] += ddelta

    # Per-core block-label alignment: sort each core's bins by chunk-need
    # profile so residual >512 groups coincide across cores (shared max).
    need = -((-L) // 128)  # ceil(L/128)
    binof = np.empty(NTOT, np.int64)
    slotof = np.empty(NTOT, np.int64)
    neworder = np.empty(NBINS, np.int64)
    for c in range(P):
        rows = np.arange(c * NBLK, (c + 1) * NBLK)
        key = np.lexsort(
            (L[rows, 3], L[rows, 2], L[rows, 1], L[rows, 0],
             need[rows, 3], need[rows, 2], need[rows, 1], need[rows, 0])
        )
        neworder[rows] = rows[key]
    members = members[neworder]
    for b in range(NBINS):
        binof[members[b]] = b
        slotof[members[b]] = np.arange(128)
    return binof, slotof


def _plan_and_pack(src, dst, w, segblk=SEGBLK, lane_sort=True):
    """Host-side routing. Returns the static schedule and per-core arrays."""
    src = np.ascontiguousarray(src.astype(np.int64))
    dst = np.ascontiguousarray(dst.astype(np.int64))
    w = np.ascontiguousarray(w.astype(np.float32))

    cls = src // CLSZ
    deg = np.zeros((NTOT, NCLS), np.int64)
    np.add.at(deg, (dst, cls), 1)
    binof, slotof = _balance_dst(deg)

    dbin = binof[dst]
    core = dbin // NBLK
    blk = dbin % NBLK
    slot = slotof[dst].astype(np.int64)
    d128 = slot.astype(np.float32)
    idxl = (src - cls * CLSZ).astype(np.int16)

    key = ((core * NBLK + blk) * NCLS + cls).astype(np.int64)
    # fine sort includes dst slot so each 128-edge chunk covers a narrow
    # contiguous slot band (R-matrix scatter path)
    order = np.argsort(key * 128 + slot, kind="stable")
    key_s = key[order]

    cnt = np.bincount(key, minlength=P * NBLK * NCLS).reshape(P, NBLK, NCLS)
    Kbq = (cnt + 127) // 128
    Kbq = Kbq.max(axis=0)                      # [NBLK, NCLS] shared schedule
    Kbq[:, 0] = np.maximum(Kbq[:, 0], 1)       # ensure start=True exists
    Kb = Kbq.sum(axis=1)                       # chunks per block
    C = int(Kb.sum())                          # total chunks per core

    # segments of segblk blocks; within a segment, chunk columns are laid out
    # class-major: [q=0: all blocks' chunks][q=1: ...]... (gather-call order)
    seg_of_blk = np.arange(NBLK) // segblk
    nseg = int(seg_of_blk.max()) + 1
    col_of = np.zeros((NBLK, NCLS), np.int64)  # first chunk col of (b, q)
    seg_base = np.zeros(nseg + 1, np.int64)    # first col of each segment
    segs = []                                  # per segment: (b0, b1, [(q, colbase, nchunks)])
    colp = 0
    for s in range(nseg):
        b0, b1 = s * segblk, min((s + 1) * segblk, NBLK)
        seg_base[s] = colp
        calls = []
        for q in range(NCLS):
            callbase = colp
            for b in range(b0, b1):
                col_of[b, q] = colp
                colp += int(Kbq[b, q])
            if colp > callbase:
                calls.append((q, callbase, colp - callbase))
        segs.append((b0, b1, calls))
    seg_base[nseg] = colp
    assert colp == C

    # group start of each (core, blk, cls) in the sorted edge list
    cntf = cnt.reshape(-1)
    gstart = np.zeros(P * NBLK * NCLS, np.int64)
    np.cumsum(cntf[:-1], out=gstart[1:])
    grank = np.arange(N_EDGES, dtype=np.int64) - gstart[key_s]

    co = core[order]
    bo = blk[order]
    qo = cls[order]
    chunkpos = col_of[bo, qo] + (grank >> 7)   # global chunk column
    # Within each (core, chunk), assign lanes in ascending-src order so each
    # gather call's address stream is locally monotonic (HBM locality). Lane
    # permutation is free: dsel/wts/R rows and idxw follow the lane mapping.
    if lane_sort:
        gidx = co * C + chunkpos
        corder = np.argsort(gidx * 200000 + src[order], kind="stable")
        starts = np.zeros(P * C + 1, np.int64)
        np.cumsum(np.bincount(gidx, minlength=P * C), out=starts[1:])
        lane = np.empty(N_EDGES, np.int64)
        lane[corder] = np.arange(N_EDGES) - starts[gidx[corder]]
    else:
        lane = (grank & 127).astype(np.int64)

    dsel = np.zeros((P, 128, C), np.float32)
    wts = np.zeros((P, 128, C), np.float32)
    idxw16 = np.zeros((P, 16, C * 8), np.int16)

    dsel[co, lane, chunkpos] = d128[order]
    wts[co, lane, chunkpos] = w[order]
    idxw16[co, lane & 15, chunkpos * 8 + (lane >> 4)] = idxl[order]

    idxw = np.tile(idxw16, (1, 8, 1))          # replicate to 128 partitions

    # block-major column order for dsel/wts (ST batch build): all chunks of a
    # block contiguous. idxw keeps the (segment, class)-major gather order.
    perm = np.empty(C, np.int64)
    blk_base = np.zeros(NBLK + 1, np.int64)
    p = 0
    for b in range(NBLK):
        blk_base[b] = p
        for q in range(NCLS):
            for i in range(int(Kbq[b, q])):
                perm[p] = col_of[b, q] + i
                p += 1
    blk_base[NBLK] = p
    dsel = dsel[:, :, perm]
    wts = wts[:, :, perm]

    # --- banded scatter matrix R -------------------------------------------
    # Per chunk (shared across cores): slot band [lo, hi]; chunk 0 of each
    # block forced full-width (its start=True matmul zeroes the psum tile).
    slot_s = slot[order]
    lo = np.full(C, 127, np.int64)
    hi = np.zeros(C, np.int64)
    np.minimum.at(lo, chunkpos, slot_s)
    np.maximum.at(hi, chunkpos, slot_s)
    empty = lo > hi
    lo[empty] = 0
    hi[empty] = 0
    first_cols = col_of[:, 0]                     # class-0 chunk 0 per block
    lo[first_cols] = 0
    hi[first_cols] = 127
    J = hi - lo + 1
    invperm = np.empty(C, np.int64)
    invperm[perm] = np.arange(C)
    Jbm = J[perm]
    roff_bm = np.zeros(C + 1, np.int64)
    np.cumsum(Jbm, out=roff_bm[1:])
    RC = int(roff_bm[C])
    R = np.zeros((P, 128, RC), np.float32)
    rcol = roff_bm[invperm[chunkpos]] + (slot_s - lo[chunkpos])
    R[co, lane, rcol] = w[order]
    rb_blk = roff_bm[blk_base]                    # [NBLK+1] first R col per blk

    meta = {
        "C": C,
        "Kbq": Kbq,
        "blk_base": blk_base,
        "col_of": col_of,
        "seg_base": seg_base,
        "segs": segs,
        "kseg_max": int((seg_base[1:] - seg_base[:-1]).max()),
        "binof": binof,
        "slotof": slotof,
        "RC": RC,
        "rlo": lo[perm],                          # block-major per-chunk lo
        "rhi": hi[perm],
        "roff_bm": roff_bm,
        "rb_blk": rb_blk,
        "segblk": segblk,
        "R": R,
    }
    return meta, dsel, wts, idxw


def unshard_output(yts, meta):
    """yts: list of per-core yt arrays [128, NBLK*128] -> full y [N_NODES, D]."""
    binof = meta["binof"][:N_NODES]
    slotof = meta["slotof"][:N_NODES]
    allyt = np.stack(yts).astype(np.float32)   # [P, 128, NBLK*128]
    core = binof // NBLK
    col = (binof % NBLK) * 128 + slotof
    return np.ascontiguousarray(allyt[core, :, col])


def _build_nc(meta, reps=1, skip_gather=False, skip_st=False, skip_mm=False,
              skip_final=False, st_bf16=True, st_any=True, nq=4, sp=False,
              st_bufs=6, g_bufs=5, meta_bf16=False, fuse_final=True, plain_dma=False,
              st_batch=True, gather_prep=False, yt_bf16=False, use_r=False,
              r_bufs=3, y_batch=False, skip_r=False):
    import concourse.bacc as bacc
    import concourse.tile as tile
    import concourse.mybir as mybir
    from drainfix_embedded import fix_excess_waits

    C = meta["C"]
    Kbq = meta["Kbq"]
    blk_base = meta["blk_base"]
    KBmax = int(Kbq.sum(axis=1).max())
    col_of = meta["col_of"]
    seg_base = meta["seg_base"]
    segs = meta["segs"]
    kseg_max = meta["kseg_max"]
    f32 = mybir.dt.float32

    nc = bacc.Bacc("TRN2", target_bir_lowering=False, debug=False, num_devices=P,
                   num_swdge_queues=nq)
    x_dt = mybir.dt.bfloat16 if st_bf16 else f32
    xtab = nc.declare_dram_parameter("xtab", [N_NODES, D], x_dt, isOutput=False)
    meta_dt = mybir.dt.bfloat16 if meta_bf16 else f32
    if use_r:
        RC = meta["RC"]
        rlo = meta["rlo"]
        rhi = meta["rhi"]
        roff_bm = meta["roff_bm"]
        rb_blk = meta["rb_blk"]
        rseg_max = max(
            int(rb_blk[b1] - rb_blk[b0]) for (b0, b1, _) in segs
        )
        rmat_d = nc.declare_dram_parameter(
            "rmat", [128, RC], mybir.dt.bfloat16, isOutput=False
        )
    else:
        dsel_d = nc.declare_dram_parameter("dsel", [128, C], meta_dt, isOutput=False)
        wts_d = nc.declare_dram_parameter("wts", [128, C], meta_dt, isOutput=False)
    idxw_d = nc.declare_dram_parameter("idxw", [128, C * 8], mybir.dt.int16, isOutput=False)
    wmat_d = nc.declare_dram_parameter("wmat", [D, D], f32, isOutput=False)
    bb_d = nc.declare_dram_parameter("bb", [D, 1], f32, isOutput=False)   # BETA*b (plain b when fuse_final)
    if fuse_final:
        alf_d = nc.declare_dram_parameter("alf", [D, 1], f32, isOutput=False)  # leaky slope
    if not fuse_final:
        cb_d = nc.declare_dram_parameter("cb", [D, 1], f32, isOutput=False)   # (C-BETA)*b
    iota_dt = mybir.dt.bfloat16 if st_bf16 else f32
    iota_d = nc.declare_dram_parameter("iotaf", [128, 128], iota_dt, isOutput=False)
    yt_dt = mybir.dt.bfloat16 if yt_bf16 else f32
    yt = nc.declare_dram_parameter("yt", [128, NBLK * 128], yt_dt, isOutput=True)

    with tile.TileContext(nc) as tc:
        with ExitStack() as ctx:
            consts = ctx.enter_context(tc.tile_pool(name="consts", bufs=1))
            metap = ctx.enter_context(tc.tile_pool(name="meta", bufs=1))
            gpool = ctx.enter_context(tc.tile_pool(name="gseg", bufs=g_bufs))
            if use_r:
                rpool = ctx.enter_context(tc.tile_pool(name="rseg", bufs=r_bufs))
            else:
                stp = ctx.enter_context(tc.tile_pool(name="st", bufs=st_bufs))
                tmpp = ctx.enter_context(tc.tile_pool(name="sttmp", bufs=2))
            evp = ctx.enter_context(tc.tile_pool(name="ev", bufs=8))
            yp = ctx.enter_context(tc.tile_pool(name="y", bufs=4 if y_batch else 8))
            pagg = ctx.enter_context(tc.tile_pool(name="pagg", bufs=5, space="PSUM"))
            pz = ctx.enter_context(tc.tile_pool(name="pz", bufs=3, space="PSUM"))

            iota_sb = consts.tile([128, 128], iota_dt)
            nc.sync.dma_start(iota_sb[:], iota_d[:])
            w_sb = consts.tile([128, 128], f32)
            nc.sync.dma_start(w_sb[:], wmat_d[:])
            bb_sb = consts.tile([128, 1], f32)
            nc.sync.dma_start(bb_sb[:], bb_d[:])
            if fuse_final:
                alf_sb = consts.tile([128, 1], f32)
                nc.sync.dma_start(alf_sb[:], alf_d[:])
            if not fuse_final:
                cb_sb = consts.tile([128, 1], f32)
                nc.sync.dma_start(cb_sb[:], cb_d[:])

            gsems = (
                [nc.alloc_semaphore(f"gsem{q}") for q in range(8)]
                if gather_prep else None
            )
            prep_ctr = [0]
            if not use_r:
                dsel_sb = metap.tile([128, C], meta_dt)
                wts_sb = metap.tile([128, C], meta_dt)
            idxw_sb = metap.tile([128, C * 8], mybir.dt.int16)

            if use_r:
                # idxw is loop-invariant: load once per dispatch, not per rep
                nc.sync.dma_start(idxw_sb[:], idxw_d[:])

            from contextlib import nullcontext
            loop_cm = tc.For_i(0, reps, 1) if reps > 1 else nullcontext()
            with loop_cm:
             for si, (b0, b1, calls) in enumerate(segs):
                if use_r:
                    break
                c0s = int(seg_base[si])
                c1s = int(seg_base[si + 1])
                nc.sync.dma_start(idxw_sb[:, c0s * 8 : c1s * 8], idxw_d[:, c0s * 8 : c1s * 8])
                if not use_r:
                    nc.sync.dma_start(dsel_sb[:, c0s:c1s], dsel_d[:, c0s:c1s])
                    nc.sync.dma_start(wts_sb[:, c0s:c1s], wts_d[:, c0s:c1s])

             for (b0, b1, calls) in segs:
                if use_r:
                    rb0 = int(rb_blk[b0])
                    rb1 = int(rb_blk[b1])
                    rtile = rpool.tile([128, rseg_max], mybir.dt.bfloat16, tag="rseg")
                    if not skip_r:
                        nc.sync.dma_start(rtile[:, : rb1 - rb0], rmat_d[:, rb0:rb1])
                    if y_batch:
                        ys = yp.tile(
                            [128, (b1 - b0) * 128], yt_dt, tag="ys"
                        )

                segc0 = int(col_of[b0, 0])           # first col of this segment
                kseg = int(sum(n for (_, _, n) in calls))
                g = gpool.tile([128, kseg_max * 128], x_dt, tag="gseg")
                g3 = g[:].rearrange("p (k d) -> p k d", d=D)
                for ci, (q, callbase, nch) in enumerate(calls):
                    if skip_gather:
                        break
                    lc0 = callbase - segc0
                    nidx = nch * 128
                    if plain_dma:
                        nc.sync.dma_start(
                            g3[:, lc0 : lc0 + nch, :],
                            xtab[0 : nch * 128, :].rearrange(
                                "(p a) d -> p a d", a=nch
                            ),
                        )
                        continue
                    if gather_prep:
                        nc.gpsimd.dma_gather(
                            g3[:, lc0 : lc0 + nch, :],
                            xtab[q * CLSZ : (q + 1) * CLSZ, :],
                            idxw_sb[:, callbase * 8 : (callbase + nch) * 8],
                            nidx,
                            nidx,
                            D,
                            single_packet=sp,
                            queue_num=ci % nq,
                            prepare_only=True,
                            sem=gsems[prep_ctr[0] % 8],
                        )
                        prep_ctr[0] += 1
                        nc.gpsimd.trigger_dma(count=None, queue_num=ci % nq)
                    else:
                        nc.gpsimd.dma_gather(
                            g3[:, lc0 : lc0 + nch, :],
                            xtab[q * CLSZ : (q + 1) * CLSZ, :],
                            idxw_sb[:, callbase * 8 : (callbase + nch) * 8],
                            nidx,
                            nidx,
                            D,
                            single_packet=sp,
                            queue_num=ci % nq,
                        )
                for b in range(b0, b1):
                    pa = None if skip_mm else pagg.tile([128, 128], f32, tag="pagg")
                    nchunks_b = int(Kbq[b].sum())
                    st_dt = mybir.dt.bfloat16 if st_bf16 else f32
                    if use_r:
                        kb0 = int(blk_base[b])
                        done = 0
                        for q in range(NCLS):
                            for i in range(int(Kbq[b, q])):
                                cg = int(col_of[b, q]) + i
                                cl = cg - segc0
                                p = kb0 + done            # block-major chunk pos
                                lo = int(rlo[p])
                                hi = int(rhi[p])
                                ro = int(roff_bm[p]) - rb0
                                if not skip_mm:
                                    nc.tensor.matmul(
                                        out=pa[:, lo : hi + 1],
                                        lhsT=g3[:, cl, :],
                                        rhs=rtile[:, ro : ro + hi - lo + 1],
                                        start=(done == 0),
                                        stop=(done == nchunks_b - 1),
                                    )
                                done += 1
                    elif st_batch and not skip_st:
                        kb0 = int(blk_base[b])
                        kbn = int(blk_base[b + 1]) - kb0
                        stt = stp.tile([128, KBmax * 128], st_dt, tag="st")
                        st3 = stt[:].rearrange("p (k d) -> p k d", d=128)
                        tmp = tmpp.tile([128, KBmax * 128], st_dt, tag="sttmp")
                        tmp3 = tmp[:].rearrange("p (k d) -> p k d", d=128)
                        iota_bc = iota_sb[:].unsqueeze(1).broadcast_to(
                            [128, kbn, 128]
                        )
                        dsel_bc = dsel_sb[:, kb0 : kb0 + kbn].unsqueeze(
                            2
                        ).broadcast_to([128, kbn, 128])
                        wts_bc = wts_sb[:, kb0 : kb0 + kbn].unsqueeze(
                            2
                        ).broadcast_to([128, kbn, 128])
                        nc.vector.tensor_tensor(
                            tmp3[:, :kbn, :], iota_bc, dsel_bc,
                            op=mybir.AluOpType.is_equal,
                        )
                        nc.vector.tensor_tensor(
                            st3[:, :kbn, :], tmp3[:, :kbn, :], wts_bc,
                            op=mybir.AluOpType.mult,
                        )
                    if use_r:
                        if skip_final or skip_mm:
                            continue
                        aggT = evp.tile([128, 128], f32, tag="ev")
                        nc.scalar.copy(aggT[:], pa[:])
                        z = pz.tile([128, 128], f32, tag="pz")
                        nc.tensor.matmul(
                            out=z[:], lhsT=w_sb[:], rhs=aggT[:], start=True,
                            stop=True,
                        )
                        if y_batch:
                            yb = ys[:, (b - b0) * 128 : (b - b0 + 1) * 128]
                        else:
                            ybt = yp.tile([128, 128], yt_dt, tag="yb")
                            yb = ybt[:]
                        nc.scalar.activation(
                            yb,
                            z[:],
                            mybir.ActivationFunctionType.Prelu,
                            bias=bb_sb[:],
                            scale=1.0,
                            alpha=alf_sb[:],
                        )
                        if not y_batch:
                            nc.sync.dma_start(
                                yt[:, b * 128 : (b + 1) * 128], yb
                            )
                        elif b == b1 - 1:
                            nc.sync.dma_start(
                                yt[:, b0 * 128 : b1 * 128], ys[:]
                            )
                        continue
                    done = 0
                    for q in range(NCLS):
                        for i in range(int(Kbq[b, q])):
                            cg = int(col_of[b, q]) + i   # global col
                            cl = cg - segc0              # col in segment tile
                            if skip_st:
                                st = iota_sb[:]
                            elif st_batch:
                                st = st3[:, done, :]
                            else:
                                sti = stp.tile([128, 128], st_dt, tag="st")
                                eng = nc.any if st_any else nc.vector
                                eng.tensor_scalar(
                                    sti[:],
                                    iota_sb[:],
                                    dsel_sb[:, cg : cg + 1],
                                    wts_sb[:, cg : cg + 1],
                                    op0=mybir.AluOpType.is_equal,
                                    op1=mybir.AluOpType.mult,
                                )
                                st = sti[:]
                            if skip_mm:
                                done += 1
                                continue
                            nc.tensor.matmul(
                                out=pa[:],
                                lhsT=g3[:, cl, :],
                                rhs=st,
                                start=(done == 0),
                                stop=(done == nchunks_b - 1),
                            )
                            done += 1
                    if skip_final or skip_mm:
                        continue
                    aggT = evp.tile([128, 128], f32, tag="ev")
                    nc.scalar.copy(aggT[:], pa[:])
                    z = pz.tile([128, 128], f32, tag="pz")
                    nc.tensor.matmul(
                        out=z[:], lhsT=w_sb[:], rhs=aggT[:], start=True, stop=True
                    )
                    if fuse_final:
                        # y = BETA*z + (C-BETA)*relu(z) == Lrelu_{BETA/C'}(z) for
                        # BETA=0.5, C=1.0, with z = agg + b folded in as bias.
                        yb = yp.tile([128, 128], yt_dt, tag="yb")
                        nc.scalar.activation(
                            yb[:],
                            z[:],
                            mybir.ActivationFunctionType.Prelu,
                            bias=bb_sb[:],
                            scale=1.0,
                            alpha=alf_sb[:],
                        )
                    else:
                        t1 = yp.tile([128, 128], f32, tag="t1")
                        nc.scalar.activation(
                            t1[:],
                            z[:],
                            mybir.ActivationFunctionType.Relu,
                            bias=cb_sb[:],
                            scale=(C_CONST - BETA),
                        )
                        t2 = yp.tile([128, 128], f32, tag="t2")
                        nc.vector.tensor_scalar(
                            t2[:],
                            z[:],
                            BETA,
                            bb_sb[:],
                            op0=mybir.AluOpType.mult,
                            op1=mybir.AluOpType.add,
                        )
                        yb = yp.tile([128, 128], f32, tag="yb")
                        nc.vector.tensor_tensor(
                            yb[:], t1[:], t2[:], op=mybir.AluOpType.add
                        )
                    nc.sync.dma_start(yt[:, b * 128 : (b + 1) * 128], yb[:])

    nc.compile()
    if gather_prep:
        _retarget_dmasw_waits(nc, gsems)
    fix_excess_waits(nc)
    return nc


def _retarget_dmasw_waits(nc, gsems):
    """Consumers of gen_mode==1 gather preps wait on Tile's DMASW lane sems,
    but the transfer completion posts to the prep's sem= semaphore. Lane
    round-robin (i%8) mirrors our gsem assignment, so rewrite every wait on
    DMASW{k}_* to gsem{k} with the same value."""
    import re
    ids = {}
    names = {}
    for k, h in enumerate(gsems):
        ids[k] = h.num
        names[k] = f"gsem{k}"
    pat = re.compile(r"^DMASW(\d)_")
    def walk(block):
        for ins in block.instructions:
            si = getattr(ins, "sync_info", None)
            if si is None:
                continue
            for u in list(si.on_wait) + list(si.on_update):
                m = pat.match(u.ant_name or "")
                if m:
                    k = int(m.group(1))
                    u.id = ids[k]
                    u.ant_name = names[k]
        for sub in getattr(block, "blocks", []) or []:
            walk(sub)
    for fn in nc.m.functions:
        for b in fn.blocks:
            walk(b)


def make_in_maps(dsel, wts, idxw, x, W, b, ncores=P, meta_bf16=False,
                 fuse_final=True, st_bf16=True, meta=None):
    import ml_dtypes
    x = np.asarray(x, np.float32)
    W = np.asarray(W, np.float32)
    b = np.asarray(b, np.float32)
    x_cast = x.astype(ml_dtypes.bfloat16) if st_bf16 else x
    iota_dt = ml_dtypes.bfloat16 if st_bf16 else np.float32
    iota = np.tile(np.arange(128, dtype=np.float32), (128, 1)).astype(iota_dt)
    meta_dt = ml_dtypes.bfloat16 if meta_bf16 else np.float32
    in_maps = []
    for c in range(ncores):
        m = {
            "xtab": x_cast,
            "dsel": dsel[c].astype(meta_dt),
            "wts": wts[c].astype(meta_dt),
            "idxw": idxw[c],
            "wmat": W,
            "iotaf": iota,
        }
        if meta is not None and "R" in meta:
            m["rmat"] = meta["R"][c].astype(ml_dtypes.bfloat16)
        if fuse_final:
            m["bb"] = b.reshape(D, 1).astype(np.float32)
            m["alf"] = np.full((D, 1), BETA, np.float32)
        else:
            m["bb"] = (BETA * b).reshape(D, 1).astype(np.float32)
            m["cb"] = ((C_CONST - BETA) * b).reshape(D, 1).astype(np.float32)
        in_maps.append(m)
    return in_maps


BEST_CFG = dict(use_r=True, y_batch=True, yt_bf16=True, g_bufs=4, r_bufs=2)


def kernel(x, edge_index, edge_weight, W, b):
    x = np.asarray(x, np.float32)
    edge_index = np.asarray(edge_index)
    edge_weight = np.asarray(edge_weight, np.float32)
    W = np.asarray(W, np.float32)
    b = np.asarray(b, np.float32)

    meta, dsel, wts, idxw = _plan_and_pack(edge_index[0], edge_index[1], edge_weight)
    nc = _build_nc(meta, **BEST_CFG)

    from concourse.bass_utils import run_bass_kernel_spmd

    in_maps = make_in_maps(dsel, wts, idxw, x, W, b, meta=meta)
    res = run_bass_kernel_spmd(nc, in_maps, list(range(P)))
    return unshard_output([res.results[c]["yt"] for c in range(P)], meta)


# ---------------------------------------------------------------------------
# Embedded walrus workaround (kernel.py must be self-contained): split excess
# sem waits onto preceding NoOps — this walrus build rejects >1 sync wait on
# Drain and on the extended DMA instructions.
import sys as _sys
import types as _types

_dfx_src = '''
import concourse.mybir as mybir

LIMIT_DEFAULT = 1
LIMIT_BY_TYPE = {mybir.InstDrain: 1}


def fix_excess_waits(nc):
    fixed = 0

    def limit_for(ins):
        for t, lim in LIMIT_BY_TYPE.items():
            if isinstance(ins, t):
                return lim
        return LIMIT_DEFAULT

    def walk(block):
        nonlocal fixed
        insts = block.instructions
        i = 0
        while i < len(insts):
            ins = insts[i]
            si = getattr(ins, "sync_info", None)
            lim = limit_for(ins)
            if si is not None and len(si.on_wait) > lim:
                waits = list(si.on_wait)
                excess, keep = waits[:-lim], waits[-lim:]
                pos = i
                for j in range(0, len(excess), LIMIT_DEFAULT):
                    nop = mybir.InstNoOp(name=f"{ins.name}_xw{j}", ins=[], outs=[])
                    nop.engine = ins.engine
                    nop.sync_info = mybir.SyncInfo(
                        on_wait=excess[j : j + LIMIT_DEFAULT], on_update=[]
                    )
                    try:
                        nc.register_instruction(nop)
                    except Exception:
                        pass
                    insts.insert(pos, nop)
                    pos += 1
                    i += 1
                si.on_wait = keep
                fixed += 1
            i += 1
        for sub in getattr(block, "blocks", []) or []:
            walk(sub)

    for fn in nc.m.functions:
        for b in fn.blocks:
            walk(b)
    return fixed
'''

_mod = _types.ModuleType("drainfix_embedded")
exec(_dfx_src, _mod.__dict__)
_sys.modules["drainfix_embedded"] = _mod

